# revision 26
# baseline (speedup 1.0000x reference)
"""Multi-head attention on 8 TRN2 NeuronCores (data/head-parallel).

Problem: B=4 H=16 S=2048 D=64 fp32 attention, out = softmax(Q K^T / sqrt(D)) V.
B*H = 64 (batch, head) pairs are sharded 8 per core; each core runs the same
NEFF over its own 8 heads, no collectives. ~233us HW exec (baseline: 287us).

Design (what each part buys):
  - QK^T uses 2-way PE row tiling (64x128 mode, array tiles T0/T8). The
    d=64 contraction fills only half the 128-row PE array, so k-tile 2j's
    K^T sits in rows 0:64 and k-tile 2j+1's in rows 64:128; Q^T is
    host-duplicated into both SBUF partition halves and the two score
    matmuls stream concurrently => ~2x QK throughput. One stationary load
    covers 1024 q columns (both 512-wide sub-blocks) to amortize the
    LDWEIGHTS that cannot hide behind a same-row-group matmul.
  - exp runs on two engines (ACT alone would take ~230us for the S^2
    scores at 1 elem/cycle/lane): ACT computes exact exp for 9 of 16
    score tiles per super-block; the other 7 are a one-instruction
    Schraudolph exponential on the Vector engine: i16 = round(score *
    128*log2(e)/sqrt(D) + 16248.5) IS the bf16 bit pattern of
    exp(score/sqrt(D)) to ~1.8% rms, and the PV matmul reads it through a
    bitcast view. Softmax's scale invariance cancels the common-mode part
    of that error; measured end-to-end rel err ~1.0e-2 vs the 2e-2 gate.
    Assignments alternate (ACT/DVE) so neither queue gets a serial run.
  - The PE executes the Tile-scheduled stream essentially in emission
    order, so the kernel software-pipelines explicitly: PV for
    super-block g-1 is interleaved 8 matmuls per 2 QK groups of
    super-block g. The PV operands (E tiles of g-1) are always ready, so
    the PE's PSUM-slot waits on the exp engines are covered with useful
    work. (Walrus limits row-tiled matmuls' sync waits, which forces
    exactly this one-super-block lag: shorter lags make Tile emit 2-wait
    tiled matmuls that fail NEFF codegen.)
  - V gets a ones column appended, so PV's PSUM accumulator is the
    unnormalized output transpose [65, q] with softmax denominators in
    row 64. DVE copies it to SBUF and it is DMAd out as-is; the division
    and [d, q] -> [q, d] transpose happen on the host. This removes the
    PE transposes, the DVE reciprocal/scale chain, and a PSUM bank.
  - PSUM budget (8 banks): 3 x score tiles [128,1024]f32 (2 banks each) +
    2 x PV accumulators [128,512]f32 = 8.
  - kt/vp are host-packed so every DMA is contiguous per partition; the
    first head's loads avoid the Scalar queue, whose head is occupied by
    walrus's prefetched ACT exp-table load.
"""

import math
from contextlib import ExitStack

import ml_dtypes
import numpy as np

import concourse.bass as bass
import concourse.bacc as bacc
import concourse.tile as tile
import concourse.mybir as mybir
from concourse.bass_utils import run_bass_kernel_spmd

B, H, S, D = 4, 16, 2048, 64
N_CORES = 8
HPC = B * H // N_CORES     # heads per core
NPAIR = 8                  # k-tile pairs (16 k-tiles of 128)
QB = 512                   # q sub-block (one PSUM bank of scores per k-tile)
SB = 1024                  # q super-block (one stationary load per k-tile)
NSB = S // SB
DT = mybir.dt

# Schraudolph-in-bf16-bit-space constants: exp(score/8) ~= bits of
# int16(score * EXP_A + EXP_B). EXP_B calibrated for round-to-nearest.
EXP_A = 128.0 * 1.4426950408889634 / 8.0
EXP_B = 16248.5
# (pair, sub-block) slots whose exp runs on DVE instead of ACT (7 of 16).
DVE_SLOTS = ((1, 1), (2, 0), (3, 1), (4, 0), (5, 1), (6, 0), (7, 1))

_BUILT = {}


class _Bacc(bacc.Bacc):
    """Bacc with the move-matmul-waits-to-ldweights pass disabled: keeping
    waits on the matmul (not its LDWEIGHTS) lets the PE queue pull weight
    loads ahead of in-flight matmuls, hiding most of the LDW cost, and
    avoids walrus folding LDW waits into the matmul's sync-wait budget."""

    def move_matmul_waits_to_ldweights(self):
        pass


def _head(nc, pools, scale, qt_d, kt_d, vp_d, h):
    """Emit head h's loads + per-super-block QK/exp. Yields None at each
    2-QK-group boundary (interleave point for the previous super-block's
    PV) and (vp, ets, q0) when a super-block's score tiles are emitted."""
    (stage, epool, spool, ps_st, ps_ot) = pools
    # qt rows 0:64 / 64:128 both hold Q^T. kt packs k-tile pairs: rows
    # 0:64 = K^T of tile 2j, rows 64:128 = K^T of tile 2j+1. vp is
    # partition-major: vp[p, t, e] = V'[t*128 + p, e].
    qt = stage.tile([128, S], DT.bfloat16, tag="qt")
    kt = stage.tile([128, NPAIR * 128], DT.bfloat16, tag="kt")
    vp = stage.tile([128, 2 * NPAIR, 128], DT.bfloat16, tag="vp")
    if h == 0:
        # first QK group needs only kt pair 0 (32KB) + qt[0:512]; land
        # those first so the PE starts ~2us earlier.
        nc.sync.dma_start(out=kt[:, 0:128], in_=kt_d[h][:, 0:128])
        nc.sync.dma_start(out=qt[:, 0:QB], in_=qt_d[h][:, 0:QB])
        nc.sync.dma_start(out=kt[:, 128:], in_=kt_d[h][:, 128:])
        nc.gpsimd.dma_start(out=qt[:, QB:SB], in_=qt_d[h][:, QB:SB])
        nc.gpsimd.dma_start(out=qt[:, SB:], in_=qt_d[h][:, SB:])
    else:
        nc.gpsimd.dma_start(out=kt, in_=kt_d[h])
        for j in range(2):
            half = slice(j * (S // 2), (j + 1) * (S // 2))
            nc.gpsimd.dma_start(out=qt[:, half], in_=qt_d[h][:, half])
    nc.gpsimd.dma_start(out=vp,
                        in_=vp_d[h].rearrange("p (t e) -> p t e", e=128))

    for c in range(NSB):
        q0 = c * SB
        ets = []
        for j in range(NPAIR):
            if j % 2 == 0:
                yield None
            st0 = ps_st.tile([128, 2 * QB], DT.float32, tag="st")
            st1 = ps_st.tile([128, 2 * QB], DT.float32, tag="st")
            # T0: k-tile 2j via rows 0:64; T8: k-tile 2j+1 via rows
            # 64:128, concurrent. Sub-blocks fill each st tile's 2 banks.
            for s, st in enumerate((st0, st1)):
                nc.tensor.matmul(
                    st[:, 0:QB],
                    lhsT=kt[0:64, j * 128 : (j + 1) * 128],
                    rhs=qt[0:64, q0 + s * QB : q0 + (s + 1) * QB],
                    start=True, stop=True,
                )
            for s, st in enumerate((st0, st1)):
                nc.tensor.matmul(
                    st[:, QB : 2 * QB],
                    lhsT=kt[64:128, j * 128 : (j + 1) * 128],
                    rhs=qt[64:128, q0 + s * QB : q0 + (s + 1) * QB],
                    start=True, stop=True,
                )
            for s, st in enumerate((st0, st1)):
                et = epool.tile([128, 2 * QB], DT.bfloat16, tag=f"et{j}_{s}")
                if (j, s) in DVE_SLOTS:
                    nc.vector.tensor_scalar(
                        et.bitcast(DT.int16), st, EXP_A, EXP_B,
                        mybir.AluOpType.mult, mybir.AluOpType.add,
                    )
                else:
                    nc.scalar.activation(
                        out=et, in_=st,
                        func=mybir.ActivationFunctionType.Exp, scale=scale,
                    )
                ets.append(et)
        yield (vp, list(ets), q0)


def _pv_gen(nc, pools, vp, ets, o_ap, q0):
    """PV for one super-block, yielding every 8 matmuls so the driver can
    interleave it into the next super-block's QK groups. The ones column
    of V makes PSUM row 64 the softmax denominator."""
    (stage, epool, spool, ps_st, ps_ot) = pools
    k = 0
    for s in range(2):
        ot = ps_ot.tile([128, QB], DT.float32, tag="ot")
        for j in range(NPAIR):
            for half in range(2):
                t = 2 * j + half
                nc.tensor.matmul(
                    ot,
                    lhsT=vp[:, t, :],
                    rhs=ets[2 * j + s][:, half * QB : (half + 1) * QB],
                    start=(t == 0), stop=(t == 2 * NPAIR - 1),
                )
                k += 1
                if k % 8 == 0:
                    yield
        # unnormalized out^T: rows 0:64 numerator, row 64 denominator.
        ots = spool.tile([D + 1, QB], DT.float32, tag="ots")
        nc.vector.tensor_copy(out=ots, in_=ot[0 : D + 1, :])
        nc.sync.dma_start(out=o_ap[:, q0 + s * QB : q0 + (s + 1) * QB],
                          in_=ots)


def build_graph(scale: float, heads: int = HPC):
    nc = _Bacc("TRN2", target_bir_lowering=False, debug=False,
               num_devices=N_CORES)
    qt_d = nc.dram_tensor("QT", [heads, 128, S], DT.bfloat16,
                          kind="ExternalInput").ap()
    kt_d = nc.dram_tensor("KT", [heads, 128, NPAIR * 128], DT.bfloat16,
                          kind="ExternalInput").ap()
    vp_d = nc.dram_tensor("VP", [heads, 128, 2 * NPAIR * 128], DT.bfloat16,
                          kind="ExternalInput").ap()
    o_d = nc.dram_tensor("out", [heads, D + 1, S], DT.float32,
                         kind="ExternalOutput").ap()

    with tile.TileContext(nc) as tc, ExitStack() as ctx:
        stage = ctx.enter_context(tc.tile_pool(name="stage", bufs=2))
        epool = ctx.enter_context(tc.tile_pool(name="epool", bufs=2))
        spool = ctx.enter_context(tc.tile_pool(name="spool", bufs=4))
        ps_st = ctx.enter_context(tc.tile_pool(name="ps_st", bufs=3,
                                               space="PSUM"))
        ps_ot = ctx.enter_context(tc.tile_pool(name="ps_ot", bufs=2,
                                               space="PSUM"))

        pools = (stage, epool, spool, ps_st, ps_ot)
        prev = None
        for h in range(heads):
            for item in _head(nc, pools, scale, qt_d, kt_d, vp_d, h):
                if item is None:
                    if prev is not None:
                        next(prev, None)   # 8 PV MMs of super-block g-1
                    continue
                if prev is not None:
                    for _ in prev:         # PV tail + copies + stores
                        pass
                prev = _pv_gen(nc, pools, item[0], item[1], o_d[h], item[2])
        if prev is not None:
            for _ in prev:                 # drain the final super-block
                pass
    nc.compile()
    return nc


def _get_nc(scale: float):
    key = round(float(scale), 9)
    if key not in _BUILT:
        _BUILT[key] = build_graph(float(scale))
    return _BUILT[key]


def shard_inputs(Q, K, V):
    """Host-side prep: shard heads across cores; build qt (Q^T duplicated
    into both 64-row halves), kt (k-tile pairs packed for row tiling), vp
    (V plus a ones column, partition-major so every DMA is contiguous)."""
    bf16 = ml_dtypes.bfloat16
    qs = np.asarray(Q, dtype=np.float32).reshape(B * H, S, D)
    ks = np.asarray(K, dtype=np.float32).reshape(B * H, S, D)
    vs = np.asarray(V, dtype=np.float32).reshape(B * H, S, D)
    qtT = qs.transpose(0, 2, 1).astype(bf16)           # [BH, D, S]
    qt = np.concatenate([qtT, qtT], axis=1)            # [BH, 128, S]
    ktT = ks.transpose(0, 2, 1).astype(bf16)           # [BH, D, S]
    ktv = ktT.reshape(B * H, D, NPAIR, 2, 128)
    kt = np.empty((B * H, 128, NPAIR, 128), dtype=bf16)
    kt[:, :D] = ktv[:, :, :, 0, :]                     # rows 0:64 <- 2j
    kt[:, D:] = ktv[:, :, :, 1, :]                     # rows 64:128 <- 2j+1
    kt = kt.reshape(B * H, 128, NPAIR * 128)
    vpb = np.zeros((B * H, S, 128), dtype=bf16)
    vpb[:, :, :D] = vs.astype(bf16)
    vpb[:, :, D] = np.float32(1.0)
    # vp[bh, p, t*128 + e] = V'[bh, t*128 + p, e]
    vp = (vpb.reshape(B * H, 2 * NPAIR, 128, 128)
          .transpose(0, 2, 1, 3).reshape(B * H, 128, 2 * NPAIR * 128))
    in_maps = []
    for c in range(N_CORES):
        sl = slice(c * HPC, (c + 1) * HPC)
        in_maps.append({
            "QT": np.ascontiguousarray(qt[sl]),
            "KT": np.ascontiguousarray(kt[sl]),
            "VP": np.ascontiguousarray(vp[sl]),
        })
    return in_maps


def kernel(Q, K, V, d_k, **run_kwargs):
    scale = 1.0 / math.sqrt(float(d_k))
    nc = _get_nc(scale)
    in_maps = shard_inputs(Q, K, V)
    res = run_bass_kernel_spmd(nc, in_maps, core_ids=list(range(N_CORES)),
                               **run_kwargs)
    # device output is [heads, 65, S]: rows 0:64 = (sum_k p*V)^T, row 64 =
    # softmax denominator. Normalize + transpose on the host.
    outs = []
    for r in res.results:
        o = r["out"]
        outs.append((o[:, :D, :] / o[:, D : D + 1, :]).transpose(0, 2, 1))
    out = np.concatenate(outs, axis=0).reshape(B, H, S, D)
    out = np.ascontiguousarray(out, dtype=np.float32)
    kernel.last_results = res
    return out



# revision 27
# speedup vs baseline: 1.0098x; 1.0098x over previous
"""Multi-head attention on 8 TRN2 NeuronCores (data/head-parallel).

Problem: B=4 H=16 S=2048 D=64 fp32 attention, out = softmax(Q K^T / sqrt(D)) V.
B*H = 64 (batch, head) pairs are sharded 8 per core; each core runs the same
NEFF over its own 8 heads, no collectives. ~233us HW exec (baseline: 287us).

Design (what each part buys):
  - QK^T uses 2-way PE row tiling (64x128 mode, array tiles T0/T8). The
    d=64 contraction fills only half the 128-row PE array, so k-tile 2j's
    K^T sits in rows 0:64 and k-tile 2j+1's in rows 64:128; Q^T is
    host-duplicated into both SBUF partition halves and the two score
    matmuls stream concurrently => ~2x QK throughput. One stationary load
    covers 1024 q columns (both 512-wide sub-blocks) to amortize the
    LDWEIGHTS that cannot hide behind a same-row-group matmul.
  - exp runs on two engines (ACT alone would take ~230us for the S^2
    scores at 1 elem/cycle/lane): ACT computes exact exp for 9 of 16
    score tiles per super-block; the other 7 are a one-instruction
    Schraudolph exponential on the Vector engine: i16 = round(score *
    128*log2(e)/sqrt(D) + 16248.5) IS the bf16 bit pattern of
    exp(score/sqrt(D)) to ~1.8% rms, and the PV matmul reads it through a
    bitcast view. Softmax's scale invariance cancels the common-mode part
    of that error; measured end-to-end rel err ~1.0e-2 vs the 2e-2 gate.
    Assignments alternate (ACT/DVE) so neither queue gets a serial run.
  - The PE executes the Tile-scheduled stream essentially in emission
    order, so the kernel software-pipelines explicitly: PV for
    super-block g-1 is interleaved 8 matmuls per 2 QK groups of
    super-block g. The PV operands (E tiles of g-1) are always ready, so
    the PE's PSUM-slot waits on the exp engines are covered with useful
    work. (Walrus limits row-tiled matmuls' sync waits, which forces
    exactly this one-super-block lag: shorter lags make Tile emit 2-wait
    tiled matmuls that fail NEFF codegen.)
  - V gets a ones column appended, so PV's PSUM accumulator is the
    unnormalized output transpose [65, q] with softmax denominators in
    row 64. DVE copies it to SBUF and it is DMAd out as-is; the division
    and [d, q] -> [q, d] transpose happen on the host. This removes the
    PE transposes, the DVE reciprocal/scale chain, and a PSUM bank.
  - PSUM budget (8 banks): 3 x score tiles [128,1024]f32 (2 banks each) +
    2 x PV accumulators [128,512]f32 = 8.
  - kt/vp are host-packed so every DMA is contiguous per partition; the
    first head's loads avoid the Scalar queue, whose head is occupied by
    walrus's prefetched ACT exp-table load.
"""

import math
from contextlib import ExitStack

import ml_dtypes
import numpy as np

import concourse.bass as bass
import concourse.bacc as bacc
import concourse.tile as tile
import concourse.mybir as mybir
from concourse.bass_utils import run_bass_kernel_spmd

B, H, S, D = 4, 16, 2048, 64
N_CORES = 8
HPC = B * H // N_CORES     # heads per core
NPAIR = 8                  # k-tile pairs (16 k-tiles of 128)
QB = 512                   # q sub-block (one PSUM bank of scores per k-tile)
SB = 1024                  # q super-block (one stationary load per k-tile)
NSB = S // SB
DT = mybir.dt

# Schraudolph-in-bf16-bit-space constants: exp(score/8) ~= bits of
# int16(score * EXP_A + EXP_B). EXP_B calibrated for round-to-nearest.
EXP_A = 128.0 * 1.4426950408889634 / 8.0
EXP_B = 16248.5
# (pair, sub-block) slots whose exp runs on DVE instead of ACT (7 of 16).
DVE_SLOTS = ((1, 1), (2, 0), (3, 1), (4, 0), (5, 1), (6, 0), (7, 1))

_BUILT = {}


class _Bacc(bacc.Bacc):
    """Bacc with the move-matmul-waits-to-ldweights pass disabled: keeping
    waits on the matmul (not its LDWEIGHTS) lets the PE queue pull weight
    loads ahead of in-flight matmuls, hiding most of the LDW cost, and
    avoids walrus folding LDW waits into the matmul's sync-wait budget."""

    def move_matmul_waits_to_ldweights(self):
        pass


def _head(nc, pools, scale, qt_d, kt_d, vp_d, h):
    """Emit head h's loads + per-super-block QK/exp. Yields None at each
    2-QK-group boundary (interleave point for the previous super-block's
    PV) and (vp, ets, q0) when a super-block's score tiles are emitted."""
    (stage, epool, spool, ps_st, ps_ot) = pools
    # qt rows 0:64 / 64:128 both hold Q^T. kt packs k-tile pairs: rows
    # 0:64 = K^T of tile 2j, rows 64:128 = K^T of tile 2j+1. vp is
    # partition-major: vp[p, t, e] = V'[t*128 + p, e].
    qt = stage.tile([128, S], DT.bfloat16, tag="qt")
    kt = stage.tile([128, NPAIR * 128], DT.bfloat16, tag="kt")
    vp = stage.tile([128, 2 * NPAIR, 128], DT.bfloat16, tag="vp")
    if h == 0:
        # first QK group needs only kt pair 0 (32KB) + qt[0:512]; land
        # those first so the PE starts ~2us earlier.
        nc.sync.dma_start(out=kt[:, 0:128], in_=kt_d[h][:, 0:128])
        nc.sync.dma_start(out=qt[:, 0:QB], in_=qt_d[h][:, 0:QB])
        nc.sync.dma_start(out=kt[:, 128:], in_=kt_d[h][:, 128:])
        nc.gpsimd.dma_start(out=qt[:, QB:SB], in_=qt_d[h][:, QB:SB])
        nc.gpsimd.dma_start(out=qt[:, SB:], in_=qt_d[h][:, SB:])
    else:
        nc.gpsimd.dma_start(out=kt, in_=kt_d[h])
        for j in range(2):
            half = slice(j * (S // 2), (j + 1) * (S // 2))
            nc.gpsimd.dma_start(out=qt[:, half], in_=qt_d[h][:, half])
    nc.gpsimd.dma_start(out=vp,
                        in_=vp_d[h].rearrange("p (t e) -> p t e", e=128))

    for c in range(NSB):
        q0 = c * SB
        ets = []
        for j in range(NPAIR):
            if j % 2 == 0:
                yield None
            st0 = ps_st.tile([128, 2 * QB], DT.float32, tag="st")
            st1 = ps_st.tile([128, 2 * QB], DT.float32, tag="st")
            # T0: k-tile 2j via rows 0:64; T8: k-tile 2j+1 via rows
            # 64:128, concurrent. Sub-blocks fill each st tile's 2 banks.
            for s, st in enumerate((st0, st1)):
                nc.tensor.matmul(
                    st[:, 0:QB],
                    lhsT=kt[0:64, j * 128 : (j + 1) * 128],
                    rhs=qt[0:64, q0 + s * QB : q0 + (s + 1) * QB],
                    start=True, stop=True,
                )
            for s, st in enumerate((st0, st1)):
                nc.tensor.matmul(
                    st[:, QB : 2 * QB],
                    lhsT=kt[64:128, j * 128 : (j + 1) * 128],
                    rhs=qt[64:128, q0 + s * QB : q0 + (s + 1) * QB],
                    start=True, stop=True,
                )
            for s, st in enumerate((st0, st1)):
                et = epool.tile([128, 2 * QB], DT.bfloat16, tag=f"et{j}_{s}")
                if (j, s) in DVE_SLOTS:
                    nc.vector.tensor_scalar(
                        et.bitcast(DT.int16), st, EXP_A, EXP_B,
                        mybir.AluOpType.mult, mybir.AluOpType.add,
                    )
                else:
                    nc.scalar.activation(
                        out=et, in_=st,
                        func=mybir.ActivationFunctionType.Exp, scale=scale,
                    )
                ets.append(et)
        yield (vp, list(ets), q0)


def _pv_gen(nc, pools, vp, ets, o_ap, q0):
    """PV for one super-block, yielding every 8 matmuls so the driver can
    interleave it into the next super-block's QK groups. The ones column
    of V makes PSUM row 64 the softmax denominator."""
    (stage, epool, spool, ps_st, ps_ot) = pools
    k = 0
    for s in range(2):
        ot = ps_ot.tile([128, QB], DT.float32, tag="ot")
        for j in range(NPAIR):
            for half in range(2):
                t = 2 * j + half
                # lhsT loads only V's 65 real columns (64 dims + ones;
                # cols 65:128 are zero padding): LDWEIGHTS cost scales
                # with weight columns, so this halves PV's weight-bus
                # occupancy, which is shared with QK's weight prefetch.
                nc.tensor.matmul(
                    ot[0 : D + 1, :],
                    lhsT=vp[:, t, 0 : D + 1],
                    rhs=ets[2 * j + s][:, half * QB : (half + 1) * QB],
                    start=(t == 0), stop=(t == 2 * NPAIR - 1),
                )
                k += 1
                if k % 8 == 0:
                    yield
        # unnormalized out^T: rows 0:64 numerator, row 64 denominator.
        ots = spool.tile([D + 1, QB], DT.float32, tag="ots")
        nc.vector.tensor_copy(out=ots, in_=ot[0 : D + 1, :])
        nc.sync.dma_start(out=o_ap[:, q0 + s * QB : q0 + (s + 1) * QB],
                          in_=ots)


def build_graph(scale: float, heads: int = HPC):
    nc = _Bacc("TRN2", target_bir_lowering=False, debug=False,
               num_devices=N_CORES)
    qt_d = nc.dram_tensor("QT", [heads, 128, S], DT.bfloat16,
                          kind="ExternalInput").ap()
    kt_d = nc.dram_tensor("KT", [heads, 128, NPAIR * 128], DT.bfloat16,
                          kind="ExternalInput").ap()
    vp_d = nc.dram_tensor("VP", [heads, 128, 2 * NPAIR * 128], DT.bfloat16,
                          kind="ExternalInput").ap()
    o_d = nc.dram_tensor("out", [heads, D + 1, S], DT.float32,
                         kind="ExternalOutput").ap()

    with tile.TileContext(nc) as tc, ExitStack() as ctx:
        stage = ctx.enter_context(tc.tile_pool(name="stage", bufs=2))
        epool = ctx.enter_context(tc.tile_pool(name="epool", bufs=2))
        spool = ctx.enter_context(tc.tile_pool(name="spool", bufs=4))
        ps_st = ctx.enter_context(tc.tile_pool(name="ps_st", bufs=3,
                                               space="PSUM"))
        ps_ot = ctx.enter_context(tc.tile_pool(name="ps_ot", bufs=2,
                                               space="PSUM"))

        pools = (stage, epool, spool, ps_st, ps_ot)
        prev = None
        for h in range(heads):
            for item in _head(nc, pools, scale, qt_d, kt_d, vp_d, h):
                if item is None:
                    if prev is not None:
                        next(prev, None)   # 8 PV MMs of super-block g-1
                    continue
                if prev is not None:
                    for _ in prev:         # PV tail + copies + stores
                        pass
                prev = _pv_gen(nc, pools, item[0], item[1], o_d[h], item[2])
        if prev is not None:
            for _ in prev:                 # drain the final super-block
                pass
    nc.compile()
    return nc


def _get_nc(scale: float):
    key = round(float(scale), 9)
    if key not in _BUILT:
        _BUILT[key] = build_graph(float(scale))
    return _BUILT[key]


def shard_inputs(Q, K, V):
    """Host-side prep: shard heads across cores; build qt (Q^T duplicated
    into both 64-row halves), kt (k-tile pairs packed for row tiling), vp
    (V plus a ones column, partition-major so every DMA is contiguous)."""
    bf16 = ml_dtypes.bfloat16
    qs = np.asarray(Q, dtype=np.float32).reshape(B * H, S, D)
    ks = np.asarray(K, dtype=np.float32).reshape(B * H, S, D)
    vs = np.asarray(V, dtype=np.float32).reshape(B * H, S, D)
    qtT = qs.transpose(0, 2, 1).astype(bf16)           # [BH, D, S]
    qt = np.concatenate([qtT, qtT], axis=1)            # [BH, 128, S]
    ktT = ks.transpose(0, 2, 1).astype(bf16)           # [BH, D, S]
    ktv = ktT.reshape(B * H, D, NPAIR, 2, 128)
    kt = np.empty((B * H, 128, NPAIR, 128), dtype=bf16)
    kt[:, :D] = ktv[:, :, :, 0, :]                     # rows 0:64 <- 2j
    kt[:, D:] = ktv[:, :, :, 1, :]                     # rows 64:128 <- 2j+1
    kt = kt.reshape(B * H, 128, NPAIR * 128)
    vpb = np.zeros((B * H, S, 128), dtype=bf16)
    vpb[:, :, :D] = vs.astype(bf16)
    vpb[:, :, D] = np.float32(1.0)
    # vp[bh, p, t*128 + e] = V'[bh, t*128 + p, e]
    vp = (vpb.reshape(B * H, 2 * NPAIR, 128, 128)
          .transpose(0, 2, 1, 3).reshape(B * H, 128, 2 * NPAIR * 128))
    in_maps = []
    for c in range(N_CORES):
        sl = slice(c * HPC, (c + 1) * HPC)
        in_maps.append({
            "QT": np.ascontiguousarray(qt[sl]),
            "KT": np.ascontiguousarray(kt[sl]),
            "VP": np.ascontiguousarray(vp[sl]),
        })
    return in_maps


def kernel(Q, K, V, d_k, **run_kwargs):
    scale = 1.0 / math.sqrt(float(d_k))
    nc = _get_nc(scale)
    in_maps = shard_inputs(Q, K, V)
    res = run_bass_kernel_spmd(nc, in_maps, core_ids=list(range(N_CORES)),
                               **run_kwargs)
    # device output is [heads, 65, S]: rows 0:64 = (sum_k p*V)^T, row 64 =
    # softmax denominator. Normalize + transpose on the host.
    outs = []
    for r in res.results:
        o = r["out"]
        outs.append((o[:, :D, :] / o[:, D : D + 1, :]).transpose(0, 2, 1))
    out = np.concatenate(outs, axis=0).reshape(B, H, S, D)
    out = np.ascontiguousarray(out, dtype=np.float32)
    kernel.last_results = res
    return out

